# revision 23
# baseline (speedup 1.0000x reference)
"""MultiHeadAttention Trainium2 kernel, 8-way tensor-parallel over heads.

B=4, T=2048, C=1024, H=16 heads, Dh=64. Each of the 8 NeuronCores owns 2
heads. All matmul operands are bf16 (fp32 moving operands stream at 2
cycles/element; bf16 at 1), accumulation stays fp32 in PSUM.

Structure: the attention inner loop runs S^T row-tiled (both heads
concurrently in the two 64-row halves of the PE array), exp on ScalarE,
and PV accumulated over k-tiles with a ones column carrying the softmax
denominator. k-tiles are processed in groups of two so consecutive
S-pairs and consecutive PV pairs stay back-to-back in the PE queue
(weight loads and drains pipeline; fewer S<->PV boundaries). The QKV
projection of batch b+1 and the out-projection of batch b-1 are issued
as "filler" ops between k-tile groups so the in-order PE queue works
through them inside exp-wait gaps instead of serializing whole phases.

V's bias never reaches the device: since softmax rows sum to 1,
(V+bv)@w = V@w + bv, so bv's contribution (W_out @ bv) is added on the
host along with b_out. The host sums the 8 partial outputs in fp64.
"""
import sys
sys.path.insert(0, '/opt/trn_rl_repo')
from collections import deque
import numpy as np
import ml_dtypes

import concourse.bass as bass
import concourse.mybir as mybir
import concourse.tile as tile
from concourse import bacc
from concourse.bass_utils import run_bass_kernel_spmd
from concourse.masks import make_identity

F32 = mybir.dt.float32
BF16 = mybir.dt.bfloat16
AF = mybir.ActivationFunctionType

B, T, C = 4, 2048, 1024
H, DH = 16, 64
NCORES = 8
HPC = H // NCORES          # heads per core (2)
D2 = HPC * DH              # 128, local concat dim
BT = B * T                 # 8192
NT = T // 512              # q/t tiles of 512 per batch (4)
NK = T // 128              # k tiles of 128 per batch (16)
CCH = C // 128             # contraction chunks (8)
VW = 72                    # v2 per-kt-per-head width (65 used, padded)
NTS = T // 128             # out-projection row tiles per batch (16)

_NC_CACHE = {}


def build_nc():
    nc = bacc.Bacc()

    xp = nc.dram_tensor("xp", [128, B * NT, CCH, 512], BF16, kind="ExternalInput")
    wq = nc.dram_tensor("wq", [128, CCH, D2], BF16, kind="ExternalInput")
    wk = nc.dram_tensor("wk", [128, CCH, D2], BF16, kind="ExternalInput")
    wv = nc.dram_tensor("wv", [128, CCH, D2], BF16, kind="ExternalInput")
    bq = nc.dram_tensor("bq", [D2, 1], F32, kind="ExternalInput")
    bk = nc.dram_tensor("bk", [D2, 1], F32, kind="ExternalInput")
    wo = nc.dram_tensor("wo", [128, C], BF16, kind="ExternalInput")
    y = nc.dram_tensor("y", [BT, C], F32, kind="ExternalOutput")

    with tile.TileContext(nc) as tc:
        with (
            tc.tile_pool(name="singles", bufs=1) as singles,
            tc.tile_pool(name="xin", bufs=4) as xin,
            tc.tile_pool(name="qkv", bufs=2) as qkv,
            tc.tile_pool(name="vtmp", bufs=2) as vtmp_pool,
            tc.tile_pool(name="esb", bufs=6) as esb,
            tc.tile_pool(name="rsb", bufs=2) as rsb,
            tc.tile_pool(name="osb", bufs=2) as osb,
            tc.tile_pool(name="outsb", bufs=3) as outsb,
            # 8 PSUM banks: s2 2x2 + pv 2x1 + small 2x1
            tc.tile_pool(name="s2_ps", bufs=2, space="PSUM") as s2_ps,
            tc.tile_pool(name="pv_ps", bufs=2, space="PSUM") as pv_ps,
            tc.tile_pool(name="small_ps", bufs=2, space="PSUM") as small_ps,
        ):
            ident_f = singles.tile([128, 128], F32)
            make_identity(nc, ident_f)
            warm = singles.tile([128, 512], BF16, tag="warm")
            nc.vector.memset(warm, 1.0)
            for wi in range(32):
                wps = small_ps.tile([128, 512], F32, tag="sm", name=f"warm{wi}")
                nc.tensor.matmul(out=wps, lhsT=warm[:, 0:128], rhs=warm,
                                 start=True, stop=True)
            ones16 = singles.tile([128, NK, HPC, 1], BF16)
            nc.vector.memset(ones16, 1.0)

            wq_sb = singles.tile([128, CCH, D2], BF16, tag="wq")
            wk_sb = singles.tile([128, CCH, D2], BF16, tag="wk")
            wv_sb = singles.tile([128, CCH, D2], BF16, tag="wv")
            for w_dram, w_sb in ((wq, wq_sb), (wk, wk_sb), (wv, wv_sb)):
                nc.sync.dma_start(out=w_sb, in_=w_dram[:, :, :])
            bq_sb = singles.tile([D2, 1], F32, tag="bq")
            bk_sb = singles.tile([D2, 1], F32, tag="bk")
            nc.sync.dma_start(out=bq_sb, in_=bq[:, :])
            nc.sync.dma_start(out=bk_sb, in_=bk[:, :])
            wo_sb = singles.tile([128, C], BF16, tag="wo")
            nc.sync.dma_start(out=wo_sb, in_=wo[:, :])

            # per-batch persistent tiles (pool bufs=2 keeps two batches live)
            bt = {}

            def alloc_batch(b):
                bt[b] = dict(
                    qT=qkv.tile([D2, T], BF16, tag="q", name=f"qT{b}"),
                    kT=qkv.tile([D2, T], BF16, tag="k", name=f"kT{b}"),
                    v2=qkv.tile([128, NK, HPC, VW], BF16, tag="v2",
                                name=f"v2_{b}"),
                )
                nc.vector.tensor_copy(out=bt[b]["v2"][:, :, :, DH:DH + 1],
                                      in_=ones16)

            def qkv_fillers(b):
                """Per-op closures computing qT/kT/v2 for batch b."""
                alloc_batch(b)
                qT, kT, v2 = bt[b]["qT"], bt[b]["kT"], bt[b]["v2"]
                ops = []
                for tt in range(NT):
                    t0 = tt * 512
                    cell = {}

                    def dma_op(b=b, tt=tt, cell=cell):
                        xt = xin.tile([128, CCH, 512], BF16, tag="xt", name=f"xt{b}_{tt}")
                        nc.sync.dma_start(out=xt, in_=xp[:, b * NT + tt, :, :])
                        cell["xt"] = xt
                    ops.append(dma_op)

                    for w_sb, b_sb, dest in ((wq_sb, bq_sb, qT),
                                             (wk_sb, bk_sb, kT),
                                             (wv_sb, None, None)):
                        for half in range(2):
                            def mm_op(w_sb=w_sb, b_sb=b_sb, dest=dest,
                                      half=half, t0=t0, cell=cell, b=b, tt=tt):
                                if half == 0:
                                    cell["ps"] = small_ps.tile(
                                        [128, 512], F32, tag="sm",
                                        name=f"qkvps{b}_{tt}")
                                ps, xt = cell["ps"], cell["xt"]
                                for ci in range(half * 4, half * 4 + 4):
                                    nc.tensor.matmul(
                                        out=ps, lhsT=w_sb[:, ci, :],
                                        rhs=xt[:, ci, :],
                                        start=(ci == 0), stop=(ci == CCH - 1))
                                if half == 1:
                                    if dest is not None:
                                        nc.vector.tensor_scalar_add(
                                            out=dest[:, t0:t0 + 512],
                                            in0=ps, scalar1=b_sb)
                                    else:
                                        vt = vtmp_pool.tile(
                                            [128, 512], F32, tag="vt", name=f"vt{b}_{tt}")
                                        nc.vector.tensor_copy(out=vt, in_=ps)
                                        cell["vt"] = vt
                            ops.append(mm_op)
                    for s in range(4):
                        def tr_op(s=s, tt=tt, cell=cell, v2=v2, b=b):
                            tp = small_ps.tile([128, 512], F32, tag="sm",
                                               name=f"tp{b}_{tt}_{s}")
                            nc.tensor.transpose(
                                out=tp[:, 0:128],
                                in_=cell["vt"][:, s * 128:(s + 1) * 128],
                                identity=ident_f)
                            kt = tt * 4 + s
                            nc.vector.tensor_copy(
                                out=v2[:, kt, :, 0:DH],
                                in_=tp[:, 0:128].rearrange(
                                    "p (g x) -> p g x", g=2))
                        ops.append(tr_op)
                return ops

            def oproj_fillers(b, ts_lo=0, ts_hi=NTS):
                """Per-op closures computing y partial rows for batch b."""
                oT2 = bt[b]["oT2"]
                ops = []
                for ts in range(ts_lo, ts_hi):
                    def op_op(ts=ts, oT2=oT2, b=b):
                        ot = outsb.tile([128, C], F32, tag="ot", name=f"ot{b}_{ts}")
                        for n in range(2):
                            n0 = n * 512
                            ops_ps = small_ps.tile([128, 512], F32, tag="sm",
                                                   name=f"ops{b}_{ts}_{n}")
                            nc.tensor.matmul(
                                out=ops_ps,
                                lhsT=oT2[:, ts * 128:(ts + 1) * 128],
                                rhs=wo_sb[:, n0:n0 + 512],
                                start=True, stop=True)
                            nc.vector.tensor_copy(out=ot[:, n0:n0 + 512],
                                                  in_=ops_ps)
                        nc.sync.dma_start(
                            out=y[b * T + ts * 128:b * T + (ts + 1) * 128, :],
                            in_=ot)
                    ops.append(op_op)
                return ops

            # prologue: batch 0's QKV runs un-interleaved
            for op in qkv_fillers(0):
                op()

            for b in range(B):
                fill = deque()
                if b + 1 < B:
                    fill.extend(qkv_fillers(b + 1))
                if b >= 1:
                    fill.extend(oproj_fillers(b - 1))
                n_fill = len(fill)
                issued = [0]

                bt[b]["oT2"] = osb.tile([128, T], BF16, tag="o2",
                                        name=f"oT2_{b}")
                oT2 = bt[b]["oT2"]
                qT, kT, v2 = bt[b]["qT"], bt[b]["kT"], bt[b]["v2"]

                for qt in range(NT):
                    q0 = qt * 512
                    pvs = [pv_ps.tile([DH + 1, 512], F32, tag="pv",
                                      name=f"pv{b}_{qt}_{h}")
                           for h in range(HPC)]
                    ets = []

                    def s_pair(kt):
                        s2 = s2_ps.tile([128, 1024], F32, tag="s2",
                                        name=f"s2_{b}_{qt}_{kt}")
                        for h in range(HPC):
                            hs = h * DH
                            nc.tensor.matmul(
                                out=s2[:, h * 512:(h + 1) * 512],
                                lhsT=kT[hs:hs + DH, kt * 128:(kt + 1) * 128],
                                rhs=qT[hs:hs + DH, q0:q0 + 512],
                                start=True, stop=True)
                        et = esb.tile([128, 1024], BF16, tag="et",
                                      name=f"et{b}_{qt}_{kt}")
                        nc.scalar.activation(out=et, in_=s2,
                                             func=AF.Exp, scale=0.125)
                        ets.append(et)

                    def pv_pair(kt):
                        for h in range(HPC):
                            nc.tensor.matmul(
                                out=pvs[h],
                                lhsT=v2[:, kt, h, 0:DH + 1],
                                rhs=ets[kt][:, h * 512:(h + 1) * 512],
                                start=(kt == 0), stop=(kt == NK - 1))

                    # 2-kt groups, PV lagging >=2 k-tiles: both PV pairs
                    # run back-to-back before the S-pairs, and every PV's
                    # exp input is at least two ACT slots old, so the PE
                    # queue never stalls on a fresh exp
                    next_pv = [0]

                    def pv_upto(limit, cap=2):
                        done = 0
                        while next_pv[0] <= limit and done < cap:
                            pv_pair(next_pv[0])
                            next_pv[0] += 1
                            done += 1
                    for g in range(NK // 2):
                        a, bb = 2 * g, 2 * g + 1
                        pv_upto(2 * g - 2)
                        s_pair(a)
                        s_pair(bb)
                        # drain fillers evenly across the batch's 32 groups
                        target = (n_fill * (qt * (NK // 2) + g + 1)) \
                            // (NT * (NK // 2))
                        while issued[0] < target and fill:
                            fill.popleft()()
                            issued[0] += 1
                    while next_pv[0] < NK:
                        pv_pair(next_pv[0])
                        next_pv[0] += 1
                    # evacuate PV (num rows 0:64, Z row 64), normalize
                    for h in range(HPC):
                        pvc = rsb.tile([DH + 1, 512], F32, tag="pvc")
                        nc.vector.tensor_copy(out=pvc, in_=pvs[h])
                        z = rsb.tile([1, 512], F32, tag="z")
                        nc.vector.tensor_copy(out=z, in_=pvc[DH:DH + 1, :])
                        r = rsb.tile([1, 512], F32, tag="r")
                        nc.vector.reciprocal_approx_fast(out=r, in_=z)
                        rbc = rsb.tile([DH, 512], F32, tag="rbc")
                        nc.gpsimd.partition_broadcast(rbc, r)
                        nc.vector.tensor_mul(
                            out=oT2[h * DH:(h + 1) * DH, q0:q0 + 512],
                            in0=pvc[0:DH, :], in1=rbc)
                while fill:
                    fill.popleft()()

            # epilogue: last batch's out-projection
            for op in oproj_fillers(B - 1):
                op()

    nc.compile()
    return nc


def make_in_maps(x, W_qkv, b_qkv, W_out, b_out):
    BF = ml_dtypes.bfloat16
    # x pre-tiled to the exact SBUF layout:
    # xp[p, tile, ci, c] = x[tile*512+c, ci*128+p]
    xp = np.ascontiguousarray(
        x.reshape(B * NT, 512, CCH, 128).transpose(3, 0, 2, 1)).astype(BF)
    in_maps = []
    for c in range(NCORES):
        r0 = c * D2
        def wshuf(wslice):
            # [D2, C] weight rows -> lhsT chunks [128 p, CCH, D2]
            return np.ascontiguousarray(
                wslice.T.reshape(CCH, 128, D2).transpose(1, 0, 2)).astype(BF)
        wqc = wshuf(W_qkv[r0:r0 + D2, :])
        wkc = wshuf(W_qkv[C + r0:C + r0 + D2, :])
        wvc = wshuf(W_qkv[2 * C + r0:2 * C + r0 + D2, :])
        bqc = np.ascontiguousarray(b_qkv[r0:r0 + D2].reshape(D2, 1))
        bkc = np.ascontiguousarray(b_qkv[C + r0:C + r0 + D2].reshape(D2, 1))
        woc = np.ascontiguousarray(W_out[:, r0:r0 + D2].T).astype(BF)
        in_maps.append({
            "xp": xp, "wq": wqc, "wk": wkc, "wv": wvc,
            "bq": bqc, "bk": bkc, "wo": woc,
        })
    return in_maps


def run(x, W_qkv, b_qkv, W_out, b_out, trace=False):
    if "nc" not in _NC_CACHE:
        _NC_CACHE["nc"] = build_nc()
    nc = _NC_CACHE["nc"]
    in_maps = make_in_maps(
        np.asarray(x, dtype=np.float32), np.asarray(W_qkv, dtype=np.float32),
        np.asarray(b_qkv, dtype=np.float32), np.asarray(W_out, dtype=np.float32),
        np.asarray(b_out, dtype=np.float32))
    res = run_bass_kernel_spmd(nc, in_maps, core_ids=list(range(NCORES)),
                               trace=trace)
    acc = np.zeros((BT, C), dtype=np.float64)
    for c in range(NCORES):
        acc += res.results[c]["y"]
    # b_out plus the folded V-bias contribution (softmax rows sum to 1)
    acc += (np.asarray(b_out, dtype=np.float64)
            + np.asarray(W_out, dtype=np.float64)
            @ np.asarray(b_qkv, dtype=np.float64)[2 * C:])[None, :]
    out = acc.astype(np.float32).reshape(B, T, C)
    return out, res


def kernel(x, W_qkv, b_qkv, W_out, b_out):
    out, _ = run(x, W_qkv, b_qkv, W_out, b_out, trace=False)
    return out
